# revision 1
# baseline (speedup 1.0000x reference)
"""K-center style kernel: argmax_i min_j ||A_i - B_j|| on 8 NeuronCores.

Strategy:
  - Shard A row-wise over 8 cores (6250 rows each, padded to 6272 = 49*128).
  - Host: pad B to 5120 columns (copies of one real column), sort by
    nb = ||b||^2; group into runs of G=16 sorted columns with per-group
    nb midpoint.
  - Device (per core): matmuls (bf16, fp32 PSUM) produce -2 a_i.b_j in
    4-bank PSUM tiles (chunks of 512, bank aligned); one grouped DVE
    tensor_reduce(min) per PSUM tile gives per-group minima of -2p;
    per row-tile a tiny add(nb_mid) + min-reduce yields
    m[i] ~= min_j (nb_j - 2 a_i.b_j).
  - Host: D_approx = sqrt(max(na + m, 0)); select candidate rows within
    DELTA of the max; rescore candidates exactly in float64; return
    (argmax int32, max float32).

The host rescore makes the final answer exact regardless of device
precision; the device pass only needs the true argmax inside the
candidate set. Device error sources: bf16 input rounding (|D err| ~1e-2)
+ nb grouping (~1e-2). DELTA = 0.1 is far above both.
"""

import numpy as np
import ml_dtypes

N_CORES = 8
N_TOTAL = 50000
M_B = 5000
M_PAD = 5120                              # padded B columns (10 * 512)
D_FEAT = 512
N_PER_CORE = N_TOTAL // N_CORES          # 6250
ROW_TILES = 49                            # ceil(6250/128)
N_PAD = ROW_TILES * 128                   # 6272
K_TILES = 2                               # 512 / 256 (DoubleRow: 256 K per pass)
N_CHUNK = 512                             # matmul free dim = one fp32 PSUM bank
GRP = 128                                 # B columns per min-group (sorted by nb)

DELTA = 1.0  # candidate slack in D units (covers fp8 e4m3 + grouping error)

_compiled = None


def build_program(row_tiles=ROW_TILES, m_b=M_PAD, k_tiles=K_TILES, n_chunk=N_CHUNK, grp=GRP):
    import concourse.tile as tile
    import concourse.mybir as mybir
    from concourse import bacc

    n_chunks = m_b // n_chunk
    n_groups = m_b // grp
    gpc = n_chunk // grp                 # groups per chunk
    assert m_b % n_chunk == 0 and n_chunk % grp == 0

    nc = bacc.Bacc("TRN2", target_bir_lowering=False, debug=False)
    atb = nc.dram_tensor(
        "ATB", [row_tiles, 128, 512], mybir.dt.float8e4, kind="ExternalInput"
    ).ap()
    btb = nc.dram_tensor(
        "BTB", [128, 4 * m_b], mybir.dt.float8e4, kind="ExternalInput"
    ).ap()
    nbg = nc.dram_tensor(
        "NBG", [128, n_groups], mybir.dt.float32, kind="ExternalInput"
    ).ap()
    mout = nc.dram_tensor(
        "M", [128, row_tiles], mybir.dt.float32, kind="ExternalOutput"
    ).ap()

    fp32 = mybir.dt.float32
    fp8 = mybir.dt.float8e4
    DR = mybir.MatmulPerfMode.DoubleRow
    add = mybir.AluOpType.add
    amin = mybir.AluOpType.min
    X = mybir.AxisListType.X

    # chunk groups -> one PSUM tile each; first group small so the DVE
    # drain of this row-tile starts early and finishes with the PE stream
    psgroups = []
    c = 0
    first = True
    while c < n_chunks:
        w = min(2 if first else 4, n_chunks - c)
        first = False
        psgroups.append((c, w))
        c += w

    with tile.TileContext(nc) as tc:
        with (
            tc.tile_pool(name="const", bufs=1) as cpool,
            tc.tile_pool(name="psum", bufs=2, space="PSUM") as pspool,
            tc.tile_pool(name="gm", bufs=row_tiles) as gmpool,
            tc.tile_pool(name="sfin", bufs=3) as spool,
            tc.tile_pool(name="mout", bufs=1) as mpool,
        ):
            # All of A^T resident: [128, row_tiles*512] bf16, one DMA per
            # row-tile on the sync queue (first matmul only needs piece 0).
            # DMA order tuned for startup: A row-tile 0, then the first
            # chunk-group of B^T split across both HWDGE queues, then the
            # rest of A on sync and the rest of B^T on scalar.
            a_all = cpool.tile([128, row_tiles * 512], fp8)
            bt_sb = cpool.tile([128, 4 * m_b], fp8)  # [p, kt(2), half(2), j]
            nc.sync.dma_start(out=a_all[:, 0:512], in_=atb[0])
            c0, w = psgroups[0]
            qflip = 0
            for nl in range(w):
                for kt in range(2):
                    for half in range(2):
                        lo = kt * 2 * m_b + half * m_b + (c0 + nl) * n_chunk
                        hi = lo + n_chunk
                        eng = nc.sync if qflip % 2 == 0 else nc.scalar
                        qflip += 1
                        eng.dma_start(out=bt_sb[:, lo:hi], in_=btb[:, lo:hi])
            for it in range(1, row_tiles):
                nc.sync.dma_start(
                    out=a_all[:, it * 512 : (it + 1) * 512], in_=atb[it]
                )
            for c0, w in psgroups[1:]:
                for kt in range(2):
                    for half in range(2):
                        lo = kt * 2 * m_b + half * m_b + c0 * n_chunk
                        hi = lo + w * n_chunk
                        nc.scalar.dma_start(out=bt_sb[:, lo:hi], in_=btb[:, lo:hi])
            nbg_sb = cpool.tile([128, n_groups], fp32)
            nc.scalar.dma_start(out=nbg_sb[:], in_=nbg[:])
            m_sb = mpool.tile([128, row_tiles], fp32)

            gm_tiles = [gmpool.tile([128, n_groups], fp32, tag="gm", name=f"gm{i}") for i in range(row_tiles)]
            last_c0 = psgroups[-1][0]
            for c0, w in psgroups:
                for it in range(row_tiles):
                    ps = pspool.tile([128, 4 * n_chunk], fp32)
                    bt_v = bt_sb[:].rearrange("p (kt two j) -> p kt two j", kt=2, two=2)
                    for nl in range(w):
                        n = c0 + nl
                        for kt in range(2):
                            lhsT3 = a_all[
                                :, it * 512 + kt * 256 : it * 512 + (kt + 1) * 256
                            ].rearrange("p (two f) -> p two f", two=2)
                            nc.tensor.matmul(
                                ps[:, nl * n_chunk : (nl + 1) * n_chunk],
                                lhsT=lhsT3,
                                rhs=bt_v[:, kt, :, n * n_chunk : (n + 1) * n_chunk],
                                start=(kt == 0),
                                stop=(kt == 1),
                                perf_mode=DR,
                            )
                    nc.vector.tensor_reduce(
                        out=gm_tiles[it][:, c0 * gpc : (c0 + w) * gpc],
                        in_=ps[:, : w * n_chunk].rearrange("p (a b) -> p a b", b=grp),
                        axis=X,
                        op=amin,
                    )
                    if c0 == last_c0:
                        s_sb = spool.tile([128, n_groups], fp32)
                        nc.vector.tensor_tensor(
                            out=s_sb[:], in0=gm_tiles[it][:], in1=nbg_sb[:], op=add
                        )
                        nc.vector.tensor_reduce(
                            out=m_sb[:, it : it + 1], in_=s_sb[:], axis=X, op=amin
                        )
            nc.sync.dma_start(out=mout[:], in_=m_sb[:])
    nc.compile()
    return nc


def prep_inputs(A, B):
    """A: [N, 512] f32 (full), B: [M, 512] f32. Returns atb, btb, nbg."""
    e4 = ml_dtypes.float8_e4m3
    B32 = B.astype(np.float32)
    nb32 = (B32**2).sum(axis=1)
    # pad B with copies of column 0 (distance contributions duplicate, min unchanged)
    Bp = np.concatenate([B32, np.broadcast_to(B32[0:1], (M_PAD - M_B, D_FEAT))], axis=0)
    nbp = np.concatenate([nb32, np.broadcast_to(nb32[0:1], (M_PAD - M_B,))])
    order = np.argsort(nbp, kind="stable")
    Bs = Bp[order]
    nbs = nbp[order]

    # ATB: per-core row-tile blocks [core, 49, 128p(feat%128), 4k*128i] of -2A
    Apad = np.zeros((N_CORES, N_PAD, D_FEAT), np.float32)
    Apad[:, :N_PER_CORE, :] = (-2.0 * A.astype(np.float32)).reshape(
        N_CORES, N_PER_CORE, D_FEAT
    )
    # feature index = kt*256 + half*128 + p
    atb = np.ascontiguousarray(
        Apad.reshape(N_CORES, ROW_TILES, 128, 2, 2, 128).transpose(0, 1, 5, 3, 4, 2)
    ).reshape(N_CORES, ROW_TILES, 128, 512).astype(e4)

    # BTB: [128p, kt(2), half(2), 5120j] = Bs[j, kt*256+half*128+p]
    btb = np.ascontiguousarray(
        Bs.reshape(M_PAD, 2, 2, 128).transpose(3, 1, 2, 0)
    ).reshape(128, 4 * M_PAD).astype(e4)

    # per-group nb midpoint
    g = nbs.reshape(M_PAD // GRP, GRP)
    nb_mid = ((g.min(axis=1) + g.max(axis=1)) * 0.5).astype(np.float32)
    nbg = np.ascontiguousarray(
        np.broadcast_to(nb_mid[None, :], (128, M_PAD // GRP))
    ).astype(np.float32)
    return atb, btb, nbg


def _exact_rescore(A, B, cand):
    A64 = A[cand].astype(np.float64)
    B64 = B.astype(np.float64)
    na = (A64 * A64).sum(axis=1)[:, None]
    nb = (B64 * B64).sum(axis=1)[None, :]
    sq = na - 2.0 * (A64 @ B64.T) + nb
    d = np.sqrt(np.maximum(sq, 0.0))
    return d.min(axis=1)


def kernel(A, B, _trace=False):
    from concourse.bass_utils import run_bass_kernel_spmd

    global _compiled
    if _compiled is None:
        _compiled = build_program()
    nc = _compiled

    A = np.asarray(A, np.float32)
    B = np.asarray(B, np.float32)
    atb, btb, nbg = prep_inputs(A, B)
    in_maps = [{"ATB": atb[c], "BTB": btb, "NBG": nbg} for c in range(N_CORES)]
    res = run_bass_kernel_spmd(nc, in_maps, list(range(N_CORES)), trace=_trace)

    # Gather per-core m and undo the [128, 49] (p, it) layout -> row it*128+p
    m = np.concatenate(
        [res.results[c]["M"].T.reshape(-1)[:N_PER_CORE] for c in range(N_CORES)]
    )
    na = (A.astype(np.float64) ** 2).sum(axis=1)
    d_approx = np.sqrt(np.maximum(na + m, 0.0))
    v = d_approx.max()
    cand = np.where(d_approx >= v - DELTA)[0]
    d_exact = _exact_rescore(A, B, cand)
    w = int(np.argmax(d_exact))
    idx = int(cand[w])
    val = float(d_exact[w])
    out = (np.array(idx, dtype=np.int32), np.array(val, dtype=np.float32))
    if _trace:
        return out, res
    return out



# revision 3
# speedup vs baseline: 5.8008x; 5.8008x over previous
"""K-center kernel: argmax_i min_j ||A_i - B_j|| on 8 NeuronCores.

Strategy (prune + rescue):
  Device pass over a SUBSET of B: the Ms=512 columns with smallest
  ||b||^2 (sorted ascending).  For each row i this yields
      ub_x[i] ~= min_{j in subset} (||b_j||^2 - 2 a_i . b_j)
  which upper-bounds the full min over B.  Host then scans rows in
  descending ub order, rescoring each EXACTLY (float64, full B) and
  stops as soon as the next row's ub + EPS1 cannot beat the best exact
  value seen — sound because m_i <= ub_i <= ub_dev_i + EPS1.

  Device details (per core: 6250 rows, 49 row-tiles of 128):
    - fp8 DoubleRow matmuls produce p = -2 a.b for the 512 subset
      columns in one PSUM bank per row-tile (2 matmuls, K=256 each).
    - Row-tiles alternate between two reduce engines so neither is the
      bottleneck:
        V-tiles (Vector): grouped tensor_reduce(min) over 16 groups of
          32 columns, add per-group nb midpoints, min -> m_V.
        S-tiles (Scalar): softmin via activation(Exp) with accumulate:
          S_g = sum_j exp((SHIFT - (p + nb_mid_g))/T) over spans
          [0:128],[128:512]; host recovers
          m_S = SHIFT - T*ln(S_0+S_1) <= true min (+ group/fp8 noise).
  EPS1 absorbs all device-vs-exact error (fp8 rounding, nb group
  midpoints, softmin slack); the final answer is exact because every
  returned (idx, val) comes from the float64 host rescore.
"""

import numpy as np
import ml_dtypes

N_CORES = 8
N_TOTAL = 50000
M_B = 5000
D_FEAT = 512
N_PER_CORE = N_TOTAL // N_CORES          # 6250
ROW_TILES = 49                            # ceil(6250/128)
N_PAD = ROW_TILES * 128                   # 6272
MS = 512                                  # subset size (smallest nb)
GRP_V = 32                                # V-path nb group size (16 groups)
S_SPANS = ((0, 128), (128, 512))          # S-path activation spans

SOFT_T = 2.5                              # softmin temperature (sq units)
SOFT_SHIFT = 250.0                        # exp arg shift (sq units)
EPS1 = 0.55                               # ub_dev underestimate allowance (D units)

# tile parity: it%5 in {0,3} -> Scalar softmin tile, else Vector tile
S_TILES = [it for it in range(ROW_TILES) if it % 5 in (0, 3)]   # 20
V_TILES = [it for it in range(ROW_TILES) if it % 5 not in (0, 3)]  # 29
N_S, N_V = len(S_TILES), len(V_TILES)

_compiled = None


def build_program():
    import concourse.tile as tile
    import concourse.mybir as mybir
    from concourse import bacc

    nc = bacc.Bacc("TRN2", target_bir_lowering=False, debug=False)
    atb = nc.dram_tensor(
        "ATB", [ROW_TILES, 128, 512], mybir.dt.float8e4, kind="ExternalInput"
    ).ap()
    btb = nc.dram_tensor(
        "BTB", [128, 4 * MS], mybir.dt.float8e4, kind="ExternalInput"
    ).ap()
    nbg = nc.dram_tensor(
        "NBG", [128, MS // GRP_V], mybir.dt.float32, kind="ExternalInput"
    ).ap()
    nbb = nc.dram_tensor(
        "NBB", [128, len(S_SPANS)], mybir.dt.float32, kind="ExternalInput"
    ).ap()
    mout = nc.dram_tensor(
        "MOUT", [128, N_V], mybir.dt.float32, kind="ExternalOutput"
    ).ap()
    sout = nc.dram_tensor(
        "SOUT", [128, 2 * N_S], mybir.dt.float32, kind="ExternalOutput"
    ).ap()

    fp32 = mybir.dt.float32
    fp8 = mybir.dt.float8e4
    bf16 = mybir.dt.bfloat16
    DR = mybir.MatmulPerfMode.DoubleRow
    add = mybir.AluOpType.add
    amin = mybir.AluOpType.min
    X = mybir.AxisListType.X
    Exp = mybir.ActivationFunctionType.Exp

    n_groups = MS // GRP_V

    with tile.TileContext(nc) as tc:
        with (
            tc.tile_pool(name="const", bufs=1) as cpool,
            tc.tile_pool(name="psum", bufs=6, space="PSUM") as pspool,
            tc.tile_pool(name="fin", bufs=3) as fpool,
            tc.tile_pool(name="scr", bufs=2) as scrpool,
            tc.tile_pool(name="out", bufs=1) as mpool,
        ):
            a_all = cpool.tile([128, ROW_TILES * 512], fp8)
            bt_sb = cpool.tile([128, 4 * MS], fp8)       # [p, kt(2), half(2), j]
            nbg_sb = cpool.tile([128, n_groups], fp32)
            nbb_sb = cpool.tile([128, len(S_SPANS)], fp32)
            warm_sb = cpool.tile([128, 1], fp32)
            m_sb = mpool.tile([128, N_V], fp32)
            s_sb = mpool.tile([128, 2 * N_S], fp32)

            # DMA order: A tile 0, B (split over two queues), bias consts,
            # then the rest of A in a few grouped descriptors.
            nc.sync.dma_start(out=a_all[:, 0:512], in_=atb[0])
            nc.sync.dma_start(out=bt_sb[:, 0 : 2 * MS], in_=btb[:, 0 : 2 * MS])
            nc.scalar.dma_start(out=nbg_sb[:], in_=nbg[:])
            nc.scalar.dma_start(out=nbb_sb[:], in_=nbb[:])
            nc.scalar.dma_start(
                out=bt_sb[:, 2 * MS : 4 * MS], in_=btb[:, 2 * MS : 4 * MS]
            )
            # Early dummy Exp so the activation table load (~2.7us) overlaps
            # the initial DMA instead of stalling the first real softmin.
            nc.scalar.activation(out=warm_sb[:], in_=nbb_sb[:, 0:1], func=Exp)
            for lo, hi in ((1, 13), (13, 31), (31, ROW_TILES)):
                nc.sync.dma_start(
                    out=a_all[:, lo * 512 : hi * 512].rearrange(
                        "p (g f) -> p g f", g=hi - lo
                    ),
                    in_=atb[lo:hi].rearrange("g p f -> p g f"),
                )

            bt_v = bt_sb[:].rearrange("p (kt two j) -> p kt two j", kt=2, two=2)
            vi = 0
            si = 0
            for it in range(ROW_TILES):
                ps = pspool.tile([128, 512], fp32)
                for kt in range(2):
                    lhsT3 = a_all[
                        :, it * 512 + kt * 256 : it * 512 + (kt + 1) * 256
                    ].rearrange("p (two f) -> p two f", two=2)
                    nc.tensor.matmul(
                        ps[:],
                        lhsT=lhsT3,
                        rhs=bt_v[:, kt, :, :],
                        start=(kt == 0),
                        stop=(kt == 1),
                        perf_mode=DR,
                    )
                if it in S_TILES:
                    for g, (lo, hi) in enumerate(S_SPANS):
                        scr = scrpool.tile([128, 384], bf16)
                        nc.scalar.activation(
                            out=scr[:, : hi - lo],
                            in_=ps[:, lo:hi],
                            func=Exp,
                            bias=nbb_sb[:, g : g + 1],
                            scale=-1.0 / SOFT_T,
                            accum_out=s_sb[:, 2 * si + g : 2 * si + g + 1],
                        )
                    si += 1
                else:
                    gm = fpool.tile([128, n_groups], fp32)
                    nc.vector.tensor_reduce(
                        out=gm[:],
                        in_=ps[:].rearrange("p (a b) -> p a b", b=GRP_V),
                        axis=X,
                        op=amin,
                    )
                    sm = fpool.tile([128, n_groups], fp32)
                    nc.vector.tensor_tensor(
                        out=sm[:], in0=gm[:], in1=nbg_sb[:], op=add
                    )
                    nc.vector.tensor_reduce(
                        out=m_sb[:, vi : vi + 1], in_=sm[:], axis=X, op=amin
                    )
                    vi += 1
            nc.sync.dma_start(out=mout[:], in_=m_sb[:])
            nc.sync.dma_start(out=sout[:], in_=s_sb[:])
    nc.compile()
    return nc


def prep_inputs(A, B):
    """Returns atb [8,49,128,512] fp8, btb [128,4*MS] fp8, nbg, nbb."""
    e4 = ml_dtypes.float8_e4m3
    B32 = B.astype(np.float32)
    nb = (B32.astype(np.float64) ** 2).sum(axis=1)
    order = np.argsort(nb, kind="stable")[:MS]
    Bs = B32[order]                       # [MS, 512] ascending nb
    nbs = nb[order]

    # ATB: per-core row-tile blocks [core, 49, 128p(feat%128), 512] of -2A
    Apad = np.zeros((N_CORES, N_PAD, D_FEAT), np.float32)
    Apad[:, :N_PER_CORE, :] = (-2.0 * A.astype(np.float32)).reshape(
        N_CORES, N_PER_CORE, D_FEAT
    )
    # feature index = kt*256 + half*128 + p
    atb = (
        np.ascontiguousarray(
            Apad.reshape(N_CORES, ROW_TILES, 128, 2, 2, 128).transpose(
                0, 1, 5, 3, 4, 2
            )
        )
        .reshape(N_CORES, ROW_TILES, 128, 512)
        .astype(e4)
    )

    # BTB: [128p, kt(2), half(2), j] = Bs[j, kt*256+half*128+p]
    btb = (
        np.ascontiguousarray(Bs.reshape(MS, 2, 2, 128).transpose(3, 1, 2, 0))
        .reshape(128, 4 * MS)
        .astype(e4)
    )

    # V-path per-group nb midpoints
    g = nbs.reshape(MS // GRP_V, GRP_V)
    mid_v = ((g.min(axis=1) + g.max(axis=1)) * 0.5).astype(np.float32)
    nbg = np.ascontiguousarray(
        np.broadcast_to(mid_v[None, :], (128, MS // GRP_V))
    ).astype(np.float32)

    # S-path per-span bias: (SHIFT - nb_mid_span) / T
    mids = []
    for lo, hi in S_SPANS:
        mids.append((nbs[lo:hi].min() + nbs[lo:hi].max()) * 0.5)
    bias = (SOFT_SHIFT - np.array(mids)) / SOFT_T
    nbb = np.ascontiguousarray(
        np.broadcast_to(bias[None, :].astype(np.float32), (128, len(S_SPANS)))
    ).astype(np.float32)
    return atb, btb, nbg, nbb


def _assemble_ub(res):
    """Per-core MOUT/SOUT -> ub_x for all 50000 rows (device estimate)."""
    ub = np.empty(N_TOTAL, np.float64)
    with np.errstate(divide="ignore"):
        for c in range(N_CORES):
            mo = res.results[c]["MOUT"].astype(np.float64)   # [128, N_V]
            so = res.results[c]["SOUT"].astype(np.float64)   # [128, 2*N_S]
            rows = np.empty((ROW_TILES, 128), np.float64)
            for vi, it in enumerate(V_TILES):
                rows[it] = mo[:, vi]
            for si, it in enumerate(S_TILES):
                s12 = so[:, 2 * si] + so[:, 2 * si + 1]
                rows[it] = SOFT_SHIFT - SOFT_T * np.log(s12)
            flat = rows.reshape(-1)[:N_PER_CORE]
            ub[c * N_PER_CORE : (c + 1) * N_PER_CORE] = flat
    return ub


def _scan_rescore(A, B, ub_d):
    """Exact scan in descending device-ub order with sound stop rule."""
    A64 = A.astype(np.float64)
    B64 = B.astype(np.float64)
    nb = (B64**2).sum(axis=1)[None, :]
    order = np.argsort(-ub_d, kind="stable")
    best_val = -np.inf
    best_idx = -1
    pos = 0
    BATCH = 128
    n_scanned = 0
    while pos < N_TOTAL:
        if pos >= 8 and ub_d[order[pos]] + EPS1 < best_val:
            break
        idx = order[pos : pos + BATCH]
        Ab = A64[idx]
        na = (Ab**2).sum(axis=1)[:, None]
        sq = na - 2.0 * (Ab @ B64.T) + nb
        d = np.sqrt(np.maximum(sq, 0.0)).min(axis=1)
        w = int(np.argmax(d))
        if d[w] > best_val:
            best_val = float(d[w])
            best_idx = int(idx[w])
        n_scanned += len(idx)
        pos += BATCH
    return best_idx, best_val, n_scanned


def kernel(A, B, _trace=False):
    from concourse.bass_utils import run_bass_kernel_spmd

    global _compiled
    if _compiled is None:
        _compiled = build_program()
    nc = _compiled

    A = np.asarray(A, np.float32)
    B = np.asarray(B, np.float32)
    atb, btb, nbg, nbb = prep_inputs(A, B)
    in_maps = [
        {"ATB": atb[c], "BTB": btb, "NBG": nbg, "NBB": nbb}
        for c in range(N_CORES)
    ]
    res = run_bass_kernel_spmd(nc, in_maps, list(range(N_CORES)), trace=_trace)

    ub_x = _assemble_ub(res)
    na = (A.astype(np.float64) ** 2).sum(axis=1)
    with np.errstate(invalid="ignore"):
        ub_d = np.sqrt(np.maximum(na + ub_x, 0.0))
    ub_d = np.where(np.isnan(ub_d), np.inf, ub_d)

    idx, val, n_scanned = _scan_rescore(A, B, ub_d)
    out = (np.array(idx, dtype=np.int32), np.array(val, dtype=np.float32))
    if _trace:
        return out, res, ub_d, n_scanned
    return out
